# revision 2
# baseline (speedup 1.0000x reference)
"""Trainium2 Bass kernel for nn_KFDeepLearningModel — init-barrier bypass.

Same math as kernel3 (Kalman filter collapsed to out = hist_tail @ U, last 48
steps, K=96). Scheduling changes:
  - Bass.__init__ ends with an all-engine barrier protecting its const-AP
    memsets; this kernel never reads the const APs, so a subclass skips that
    one barrier. Engines then reach kernel instructions as soon as their own
    preamble retires instead of waiting for the slowest engine (sync, whose
    NEFF-start DRAIN costs an extra ~0.6us).
  - The whole x load is one 96-descriptor DMA on the scalar (Activation)
    engine — the earliest HWDGE engine out of init — emitted ahead of the
    Block. Scalar also issues the out-DMA (gated on matmul-done; its ~1.5us
    of gen+kick still hides the DVE copy). The sync engine does nothing.
"""

import numpy as np

_B, _T = 4096, 1024
_NCORES = 8
_RPC = _B // _NCORES        # 512 rows per core
_NKEEP = 48                 # trailing timesteps kept
_K = 2 * _NKEEP             # 96 contraction / SBUF partitions
_J = 6
_ROW = _RPC + _J            # 518 f16 per partition

_compiled = None


def _build_U(Q_log, R_log):
    """U[T*2, 6] such that out[b] = (hist[b].reshape(-1) @ U).reshape(3, 2)."""
    dtype = np.float64
    F = np.array([[1, 0, 1, 0], [0, 1, 0, 1], [0, 0, 1, 0], [0, 0, 0, 1]], dtype)
    H = np.array([[1, 0, 0, 0], [0, 1, 0, 0]], dtype)
    I4 = np.eye(4, dtype=dtype)
    Q = np.exp(np.asarray(Q_log, dtype)) + 1e-6 * I4
    R = np.exp(np.asarray(R_log, dtype)) + 1e-6 * np.eye(2, dtype=dtype)

    P = 1000.0 * I4
    A = np.zeros((_T, 4, 4), dtype)
    Kg = np.zeros((_T, 4, 2), dtype)
    FT = F.T.copy()
    HT = H.T.copy()
    for t in range(_T):
        P = F @ P @ FT + Q
        S = H @ P @ HT + R
        Kt = P @ HT @ np.linalg.inv(S)
        Kg[t] = Kt
        A[t] = (I4 - Kt @ H) @ F
        P = (I4 - Kt @ H) @ P

    W = np.zeros((_T, 4, 2), dtype)
    S_t = I4.copy()
    for t in range(_T - 1, -1, -1):
        W[t] = S_t @ Kg[t]
        S_t = S_t @ A[t]
    E = np.zeros((4, 2), dtype)
    E[0, 0] = E[1, 1] = 1.0
    W[0] += S_t @ E

    G = np.zeros((6, 4), dtype)
    for k in range(3):
        for c in range(2):
            G[2 * k + c, c] = 1.0
            G[2 * k + c, c + 2] = k + 1.0
    GW = np.einsum("ja,tac->tcj", G, W)      # [T, 2, 6]
    return GW.reshape(2 * _T, _J)


def _get_compiled():
    global _compiled
    if _compiled is None:
        from contextlib import ExitStack

        import concourse.bass as bass
        import concourse.mybir as mybir

        f32 = mybir.dt.float32
        f16 = mybir.dt.float16

        class _FastBass(bass.Bass):
            """Skips the __init__-tail all-engine barrier (protects const-AP
            memsets this kernel never reads); later barriers run normally."""

            _init_done = False

            def all_engine_barrier(self, **kw):
                if self._init_done:
                    return super().all_engine_barrier(**kw)

        nc = _FastBass("TRN2", target_bir_lowering=False, debug=False,
                       enable_partition_id=False)
        nc._init_done = True

        xu = nc.dram_tensor("xu", [_K, _ROW], f16, kind="ExternalInput").ap()
        out = nc.dram_tensor("out", [_J, _RPC], f16, kind="ExternalOutput").ap()

        with ExitStack() as ctx:
            xbuf = ctx.enter_context(nc.sbuf_tensor([_K, _ROW], f16))
            obuf = ctx.enter_context(nc.sbuf_tensor([_J, _RPC], f16))
            psum = ctx.enter_context(nc.psum_tensor([_J, _RPC], f32))
            s0 = ctx.enter_context(nc.semaphore("s0"))
            s2 = ctx.enter_context(nc.semaphore("s2"))
            s4 = ctx.enter_context(nc.semaphore("s4"))

            # Pre-Block: scalar starts pulling x the moment its own preamble
            # retires — no cross-engine wait.
            nc.scalar.dma_start(out=xbuf[:], in_=xu[:]).then_inc(s0, 16)

            block = ctx.enter_context(nc.Block(no_gpsimd_drain=True))

            @block.scalar
            def _(scalar):
                # Gen + queue-kick of the out DMA (~1.5us) hides the DVE copy
                # (~0.7us): gated on matmul-done, not copy-done.
                scalar.wait_ge(s2, 1)
                scalar.dma_start(out=out[:], in_=obuf[:]).then_inc(s4, 16)

            @block.tensor
            def _(tensor):
                tensor.wait_ge(s0, 16)
                tensor.matmul(
                    psum[:],
                    xbuf[:, _RPC:_ROW],
                    xbuf[:, 0:_RPC],
                    start=True,
                    stop=True,
                ).then_inc(s2, 1)

            @block.vector
            def _(vector):
                vector.wait_ge(s2, 1)
                vector.tensor_copy(obuf[:], psum[:])

        _compiled = nc
    return _compiled


def _make_in_maps(history_obs, Q_log, R_log):
    U = _build_U(Q_log, R_log)[-_K:].astype(np.float16)          # [96, 6]
    X = np.asarray(history_obs)[:, _T - _NKEEP :, :].reshape(_B, _K)
    in_maps = []
    for c in range(_NCORES):
        Xc = X[c * _RPC : (c + 1) * _RPC].astype(np.float16)     # [512, 96]
        xu_host = np.empty((_K, _ROW), np.float16)
        xu_host[:, 0:_RPC] = Xc.T
        xu_host[:, _RPC:_ROW] = U
        in_maps.append({"xu": xu_host})
    return in_maps


def _assemble(results):
    out = np.empty((_B, _J), np.float32)
    for c in range(_NCORES):
        out[c * _RPC : (c + 1) * _RPC] = results[c]["out"].T.astype(np.float32)
    return out.reshape(_B, 3, 2)


def kernel(history_obs, Q_log, R_log):
    from concourse.bass_utils import run_bass_kernel_spmd

    nc = _get_compiled()
    in_maps = _make_in_maps(history_obs, Q_log, R_log)
    res = run_bass_kernel_spmd(nc, in_maps, list(range(_NCORES)))
    return _assemble(res.results)


def kernel_profiled(history_obs, Q_log, R_log):
    """kernel() + NTFF trace; returns (out, exec_time_ns, trace_path)."""
    from concourse.bass_utils import run_bass_kernel_spmd

    nc = _get_compiled()
    in_maps = _make_in_maps(history_obs, Q_log, R_log)
    res = run_bass_kernel_spmd(nc, in_maps, list(range(_NCORES)), trace=True)
    trace_path = res.instructions_and_trace[1] if res.instructions_and_trace else None
    return _assemble(res.results), res.exec_time_ns, trace_path
